# revision 4
# baseline (speedup 1.0000x reference)
"""BetaTCVAE loss kernel for 8 Trainium2 NeuronCores.

Math: reference computes
    kl_loss = sum(kl)
    log_qz_prob[i,j,l] = -0.5*((z_i_l - m_j_l)^2 * exp(-v_j_l) + v_j_l + LOG2PI)
    log_qz_product[i]  = sum_l logsumexp_j log_qz_prob[i,j,l]
    log_qz[i]          = logsumexp_j sum_l log_qz_prob[i,j,l]
    out = (BETA-1)*mean_i(log_qz - log_qz_product) + kl_loss

Key transform: with w = exp(-v),
    log_qz_prob[i,j,l] = a[j,l]*z2[i,l] + b[j,l]*z[i,l] + g[j,l]
      a = -w/2, b = w*m, g = -(w*m^2 + v + LOG2PI)/2, z2 = z^2
so the [i,j] arg per l is a K=3 matmul (TensorE), and the full sum over
l (for log_qz) is a K=3L matmul.

The O(B^2*L) exp work is split across two engines:
  * ScalarE tiles: native Exp activation with fused free-dim accumulate
    (accum_out) -- 1 elem/cycle/lane @1.2GHz.
  * VectorE tiles: Schraudolph exp-as-bits. Coefficients are pre-scaled
    on host so the matmul PSUM holds y = ENC_A*arg + ENC_B with
    ENC_A = 1024/ln2, ENC_B = 15360. Then round(y + SIG) clamped >= 0 IS
    the fp16 bit pattern of ~exp(arg): one DVE tensor_scalar
    (add SIG, max 0, fp32 PSUM -> int16 SBUF), then one DVE tensor_reduce
    over the bitcast-fp16 values sums over j. HW convert is
    round-to-nearest; SIG tunes away the piecewise-linear bias
    (residual log error ~4e-3 max, well inside tolerance).
ScalarE tiles decode the same y encoding via the activation's free
affine (scale=ln2/1024, bias=-ENC_B*ln2/1024), so one coefficient
tensor and one matmul pipeline feed both consumers.

Sharding: outer batch dim i split across 8 cores (256 rows each);
coefficient tensors replicated. Per-core per-partition partials are
DMA'd out and summed on host (the trivial all-reduce), together with a
closed-form constant correcting the encoding offset in log_qz.
"""

import os
import sys
from contextlib import ExitStack

import numpy as np

for _p in ("/opt/trn_rl_repo", "/root/.axon_site/_ro/trn_rl_repo"):
    if os.path.isdir(_p) and _p not in sys.path:
        sys.path.append(_p)

import concourse.bass as bass
import concourse.tile as tile
from concourse import mybir

BETA = 6.0
LOG_2PI = float(np.log(2.0 * np.pi))
F32 = mybir.dt.float32
BF16 = mybir.dt.bfloat16
F16 = mybir.dt.float16
I16 = mybir.dt.int16
AF = mybir.ActivationFunctionType
ALU = mybir.AluOpType

ENC_A = 1024.0 / float(np.log(2.0))     # y = ENC_A*arg + ENC_B
ENC_B = 15360.0                          # = 15 * 1024 (fp16 exponent bias)
ENC_C = float(np.log(2.0)) / 1024.0     # decode scale: arg = (y-ENC_B)*ENC_C
SIG = -58.9135                           # Schraudolph bias correction
N_DVE = 40                               # of 128 phase-B tiles on the DVE path
PHASEA_AT = 3                            # run phase A after this many B tiles


def build_nc(B=2048, L=64, BC=256, n_dve=N_DVE, split_waits=True):
    """Build the per-core Bass program.

    B: total batch (j dim, replicated on every core)
    L: latent dim
    BC: rows of i handled by this core
    """
    PI = 128
    assert BC % PI == 0
    nit = BC // PI
    JT = min(512, B)
    assert B % JT == 0
    njc = B // JT
    KS = 3 * L
    KC = 96 if KS % 96 == 0 else KS
    assert KS % KC == 0
    nkc = KS // KC
    scale_r = (BETA - 1.0) / float(B)
    ntiles = L * nit

    # Bresenham-spread DVE tile assignment over the flat (l, it) index.
    def is_dve(k):
        return (k + 1) * n_dve // ntiles > k * n_dve // ntiles

    nc = bass.Bass()
    zpk_d = nc.declare_dram_parameter("zpk", [nit, 3, L * PI], BF16, False)
    zs_d = nc.declare_dram_parameter("zs", [nkc, KC, BC], BF16, False)
    coefd_d = nc.declare_dram_parameter("coefd", [L, 3, B], BF16, False)
    coefs_d = nc.declare_dram_parameter("coefs", [nkc, KC, B], BF16, False)
    kld_d = nc.declare_dram_parameter("kld", [BC, L], F32, False)
    out_d = nc.declare_dram_parameter("out", [PI, 1], F32, True)

    with tile.TileContext(nc) as tc, ExitStack() as ctx:
        const_pool = ctx.enter_context(tc.tile_pool(name="const", bufs=1))
        es_pool = ctx.enter_context(tc.tile_pool(name="es", bufs=2))
        i16_pool = ctx.enter_context(tc.tile_pool(name="i16", bufs=2))
        coefl_pool = ctx.enter_context(tc.tile_pool(name="coefl", bufs=6))
        small = ctx.enter_context(tc.tile_pool(name="small", bufs=1))
        psum = ctx.enter_context(tc.tile_pool(name="psum", bufs=2, space="PSUM"))

        # --- persistent loads ---
        zpk_t = []
        for it in range(nit):
            t = const_pool.tile([PI, L * PI], BF16, tag=f"zpk{it}", name=f"zpk{it}")
            used = sorted({(l * nit + it) % 4 for l in range(L)})
            for g in used:
                nc.gpsimd.dma_start(out=t[32 * g:32 * g + 3, :], in_=zpk_d[it])
            zpk_t.append(t)
        zs_t = []
        coefs_t = []
        kl_t = []
        for k in range(nkc):
            t = const_pool.tile([KC, BC], BF16, tag=f"zs{k}", name=f"zs{k}")
            nc.sync.dma_start(out=t[:], in_=zs_d[k])
            zs_t.append(t)
            t2 = const_pool.tile([KC, B], BF16, tag=f"cs{k}", name=f"cs{k}")
            nc.sync.dma_start(out=t2[:], in_=coefs_d[k])
            coefs_t.append(t2)
        for it in range(nit):
            t = const_pool.tile([PI, L], F32, tag=f"kl{it}", name=f"klt{it}")
            nc.sync.dma_start(out=t[:], in_=kld_d[it * PI:(it + 1) * PI, :])
            kl_t.append(t)

        g_t = [small.tile([PI, L], F32, tag=f"g{it}", name=f"g{it}")
               for it in range(nit)]
        lq_t = {}
        biasb = small.tile([PI, 1], F32, tag="biasb")
        nc.gpsimd.memset(biasb[:], -ENC_B * ENC_C)

        def phase_a(it):
            # log_qz: S = sum_l y_l = ENC_A * (sum_l arg_l) + L*ENC_B
            sp = psum.tile([PI, B], F32, tag="ring", name=f"sp{it}")
            for k in range(nkc):
                lhsT = zs_t[k][:, it * PI:(it + 1) * PI]
                for jc in range(njc):
                    nc.tensor.matmul(
                        sp[:, jc * JT:(jc + 1) * JT],
                        lhsT,
                        coefs_t[k][:, jc * JT:(jc + 1) * JT],
                        start=(k == 0),
                        stop=(k == nkc - 1),
                    )
            mx = small.tile([PI, 1], F32, tag=f"mx{it}", name=f"mx{it}")
            nc.vector.tensor_reduce(mx[:], sp[:], axis=mybir.AxisListType.X,
                                    op=ALU.max)
            negmxc = small.tile([PI, 1], F32, tag=f"negmxc{it}",
                                name=f"negmxc{it}")
            nc.scalar.mul(negmxc[:], mx[:], -ENC_C)
            es = es_pool.tile([PI, B], F32, tag="es", name=f"esA{it}")
            sume = small.tile([PI, 1], F32, tag=f"sume{it}", name=f"sume{it}")
            nc.scalar.activation(es[:], sp[:], AF.Exp, bias=negmxc[:],
                                 scale=ENC_C, accum_out=sume[:])
            lq = small.tile([PI, 1], F32, tag=f"lq{it}", name=f"lq{it}")
            nc.scalar.activation(lq[:], sume[:], AF.Ln)
            mxc = small.tile([PI, 1], F32, tag=f"mxc{it}", name=f"mxc{it}")
            nc.scalar.mul(mxc[:], mx[:], ENC_C)
            nc.vector.tensor_add(lq[:], lq[:], mxc[:])
            lq_t[it] = lq
            # lq is short by L*ENC_B*ENC_C vs ln(sum_j exp(S)); host corrects.

        # --- phase B: G[i,l] = sum_j exp(arg_l[i,j]) ---
        k_flat = 0
        for l in range(L):
            cf = coefl_pool.tile([PI, B], BF16, tag="cf")
            for g in range(4):
                nc.sync.dma_start(out=cf[32 * g:32 * g + 3, :], in_=coefd_d[l])
            for it in range(nit):
                if k_flat == PHASEA_AT:
                    for it2 in range(nit):
                        phase_a(it2)
                ap = psum.tile([PI, B], F32, tag="ring")
                g = (l * nit + it) % 4
                lhsT = zpk_t[it][32 * g:32 * g + 3, l * PI:(l + 1) * PI]
                for jc in range(njc):
                    nc.tensor.matmul(
                        ap[:, jc * JT:(jc + 1) * JT],
                        lhsT,
                        cf[32 * g:32 * g + 3, jc * JT:(jc + 1) * JT],
                        start=True,
                        stop=True,
                        tile_position=(32 * g, 0),
                    )
                gcol = g_t[it][:, l:l + 1]
                if is_dve(k_flat):
                    e16 = i16_pool.tile([PI, B], I16, tag="e16")
                    nc.vector.tensor_scalar(e16[:], ap[:], SIG, 0.0,
                                            ALU.add, ALU.max)
                    nc.vector.tensor_reduce(gcol, e16[:].bitcast(F16),
                                            axis=mybir.AxisListType.X,
                                            op=ALU.add)
                else:
                    ed = es_pool.tile([PI, B], BF16, tag="ed")
                    nc.scalar.activation(ed[:], ap[:], AF.Exp,
                                         bias=biasb[:], scale=ENC_C,
                                         accum_out=gcol)
                k_flat += 1

        # --- combine per-core: r = (lq - sum_l ln G) * (BETA-1)/B ; kl sums ---
        tot = small.tile([PI, 1], F32, tag="tot")
        for it in range(nit):
            logg = small.tile([PI, L], F32, tag=f"logg{it}")
            nc.scalar.activation(logg[:], g_t[it][:], AF.Ln)
            lqp = small.tile([PI, 1], F32, tag=f"lqp{it}")
            nc.vector.tensor_reduce(lqp[:], logg[:], axis=mybir.AxisListType.X,
                                    op=ALU.add)
            r = small.tile([PI, 1], F32, tag=f"r{it}")
            nc.vector.tensor_sub(r[:], lq_t[it][:], lqp[:])
            nc.scalar.mul(r[:], r[:], scale_r)
            kls = small.tile([PI, 1], F32, tag=f"kls{it}")
            nc.vector.tensor_reduce(kls[:], kl_t[it][:], axis=mybir.AxisListType.X,
                                    op=ALU.add)
            nc.vector.tensor_add(r[:], r[:], kls[:])
            if it == 0:
                nc.vector.tensor_copy(tot[:], r[:])
            else:
                nc.vector.tensor_add(tot[:], tot[:], r[:])
        nc.sync.dma_start(out=out_d[:], in_=tot[:])

    return _split_multi_waits(nc) if split_waits else nc


def _split_multi_waits(nc):
    """Walrus (gen3 codegen) accepts at most ONE sync-wait per instruction.
    Tile's wait assignment can attach several. Split the extras onto NoOp
    instructions on the same engine immediately before the instruction —
    same-engine streams execute in order, so semantics are preserved."""
    wid = [0]

    def fix_block(b):
        new = []
        for inst in b.instructions:
            si = inst.sync_info
            if si is not None and si.on_wait and len(si.on_wait) > 1:
                for w in si.on_wait[:-1]:
                    wid[0] += 1
                    nop = mybir.InstNoOp(
                        name=f"WSPLIT-{wid[0]}",
                        engine=inst.engine,
                        sync_info=mybir.SyncInfo(on_wait=[w], on_update=[]),
                    )
                    nop.bass_nofuse = True
                    new.append(nop)
                si.on_wait = [si.on_wait[-1]]
            new.append(inst)
        b.instructions[:] = new

    for fn in nc.m.functions:
        for b in fn.blocks:
            fix_block(b)
    return nc


def make_inputs(kl, z_mean, z_logvar, z_sampled, n_cores):
    """Host-side O(B*L) prep: y-encoded coefficient tensors + shards."""
    B, L = kl.shape
    BC = B // n_cores
    PI = 128
    nit = BC // PI
    KS = 3 * L
    KC = 96 if KS % 96 == 0 else KS
    nkc = KS // KC

    kl = np.ascontiguousarray(kl, dtype=np.float32)
    m = np.asarray(z_mean, dtype=np.float32)
    v = np.asarray(z_logvar, dtype=np.float32)
    z = np.asarray(z_sampled, dtype=np.float32)

    w = np.exp(-v)
    a = ENC_A * (-0.5 * w)
    b = ENC_A * (w * m)
    g = ENC_A * (-0.5 * (w * m * m + v + LOG_2PI)) + ENC_B
    import ml_dtypes
    coefd = np.ascontiguousarray(
        np.stack([a, b, g], 0).transpose(2, 0, 1)).astype(ml_dtypes.bfloat16)  # [L, 3, B]
    coefs = np.ascontiguousarray(
        np.stack([a, b, g], 0).transpose(2, 0, 1).reshape(3 * L, B)
        .reshape(nkc, KC, B)).astype(ml_dtypes.bfloat16)  # [nkc, KC, B]

    in_maps = []
    for c in range(n_cores):
        zc = z[c * BC:(c + 1) * BC]                       # [BC, L]
        arr = np.stack([zc * zc, zc, np.ones_like(zc)], 0)  # [3, BC, L]
        zs = np.ascontiguousarray(
            arr.transpose(2, 0, 1).reshape(3 * L, BC)
            .reshape(nkc, KC, BC)).astype(ml_dtypes.bfloat16)
        arrT = arr.transpose(0, 2, 1)                     # [3, L, BC]
        zpk = np.stack(
            [arrT[:, :, it * PI:(it + 1) * PI].reshape(3, L * PI)
             for it in range(nit)], 0).astype(ml_dtypes.bfloat16)  # [nit, 3, L*PI]
        in_maps.append({
            "zpk": np.ascontiguousarray(zpk),
            "zs": zs,
            "coefd": coefd,
            "coefs": coefs,
            "kld": np.ascontiguousarray(kl[c * BC:(c + 1) * BC]),
        })
    return in_maps


_NC_CACHE = {}


def _get_nc(B, L, BC):
    key = (B, L, BC)
    if key not in _NC_CACHE:
        _NC_CACHE[key] = build_nc(B, L, BC)
    return _NC_CACHE[key]


def _enable_jax_cache():
    try:
        import jax
        jax.config.update("jax_compilation_cache_dir", "/tmp/jaxcache")
        jax.config.update("jax_persistent_cache_min_entry_size_bytes", 0)
        jax.config.update("jax_persistent_cache_min_compile_time_secs", 0)
    except Exception:
        pass


def host_total(results, B, L):
    """Sum per-core per-partition partials + encoding-offset correction."""
    total = sum(float(np.asarray(r["out"], dtype=np.float64).sum())
                for r in results)
    total -= (BETA - 1.0) * (L * ENC_B * ENC_C)
    return np.float32(total)


def kernel(kl, z_mean, z_logvar, z_sampled):
    from concourse.bass_utils import run_bass_kernel_spmd

    _enable_jax_cache()

    B, L = kl.shape
    n_cores = 8
    BC = B // n_cores
    nc = _get_nc(B, L, BC)
    in_maps = make_inputs(kl, z_mean, z_logvar, z_sampled, n_cores)
    res = run_bass_kernel_spmd(nc, in_maps, list(range(n_cores)))
    return host_total(res.results, B, L)


# revision 5
# speedup vs baseline: 1.0092x; 1.0092x over previous
"""BetaTCVAE loss kernel for 8 Trainium2 NeuronCores.

Math: reference computes
    kl_loss = sum(kl)
    log_qz_prob[i,j,l] = -0.5*((z_i_l - m_j_l)^2 * exp(-v_j_l) + v_j_l + LOG2PI)
    log_qz_product[i]  = sum_l logsumexp_j log_qz_prob[i,j,l]
    log_qz[i]          = logsumexp_j sum_l log_qz_prob[i,j,l]
    out = (BETA-1)*mean_i(log_qz - log_qz_product) + kl_loss

Key transform: with w = exp(-v),
    log_qz_prob[i,j,l] = a[j,l]*z2[i,l] + b[j,l]*z[i,l] + g[j,l]
      a = -w/2, b = w*m, g = -(w*m^2 + v + LOG2PI)/2, z2 = z^2

Coefficients are pre-scaled on host so matmul PSUM holds
y = ENC_A*arg + ENC_B (ENC_A = 1024/ln2, ENC_B = 15360): round(y) IS the
fp16 bit pattern of exp(arg) (Schraudolph).

Phase B issues BLOCK-DIAGONAL stationaries: lhsT[(l,k), (ls,is)] is
z_k[i,l] on the block diagonal, so one K=96 matmul computes args for
32 latents x 4 batch rows at once. K=96 keeps the PE array's activity
high enough for the hardware clock ramp (K=3 matmuls never leave the
~1.2GHz p-state; K=96 reach full speed), and phase B then shares the
phase-A coefficient tensors as moving data -- no per-l DMA stream.
Off-block entries get tiny +/-1e-30 noise instead of zeros to keep
switching activity up; the products (~1e-26) are harmless.

The O(B^2*L) exp work is split across engines per tile:
  * ScalarE tiles: native Exp (scale/bias decode of y) with fused
    accum_out reduction over j.
  * VectorE tiles: one tensor_scalar (add SIG, max 0) converting fp32
    PSUM -> int16 SBUF = fp16 exp bits (HW convert is round-to-nearest,
    SIG tunes away the Schraudolph bias); optionally GPSIMD halves the
    bitcast-fp16 tile (tensor_tensor add) before a VectorE tensor_reduce
    finishes the j sum.

Everything after ln(G) is a full sum, so per-partition partials
(sum_l ln G, lq per half, and h=sum kl) are DMA'd out and summed on
host along with the closed-form encoding-offset correction.
"""

import os
import sys
from contextlib import ExitStack

import numpy as np

for _p in ("/opt/trn_rl_repo", "/root/.axon_site/_ro/trn_rl_repo"):
    if os.path.isdir(_p) and _p not in sys.path:
        sys.path.append(_p)

import concourse.bass as bass
import concourse.tile as tile
from concourse import mybir

BETA = 6.0
LOG_2PI = float(np.log(2.0 * np.pi))
F32 = mybir.dt.float32
BF16 = mybir.dt.bfloat16
F16 = mybir.dt.float16
I16 = mybir.dt.int16
AF = mybir.ActivationFunctionType
ALU = mybir.AluOpType

ENC_A = 1024.0 / float(np.log(2.0))     # y = ENC_A*arg + ENC_B
ENC_B = 15360.0                          # = 15 * 1024 (fp16 exponent bias)
ENC_C = float(np.log(2.0)) / 1024.0     # decode scale: arg = (y-ENC_B)*ENC_C
SIG = -58.9135                           # Schraudolph bias correction
N_DVE = 48                               # of 128 phase-B tiles on the DVE path
PHASEA_AT = 2                            # run phase A after this many B tiles
GPSIMD_HALVE = True                      # GPSIMD halves DVE tiles before reduce
LG = 32                                  # latents per block-diag stationary
IG = 4                                   # batch rows per block-diag stationary


def build_nc(B=2048, L=64, BC=256, n_dve=N_DVE, split_waits=True):
    PI = 128
    assert LG * IG == PI and 3 * LG <= PI
    JT = min(512, B)
    njc = B // JT
    KC = 3 * LG                          # stationary contraction dim (96)
    nkc = (3 * L) // KC                  # coefficient groups (2)
    nlg = L // LG                        # latent groups (2)
    nig = BC // IG                       # i groups per latent group (64)
    ntiles = nlg * nig                   # phase-B tiles (128)
    nit = BC // PI                       # phase-A row tiles (2)

    def is_dve(k):
        return (k + 1) * n_dve // ntiles > k * n_dve // ntiles

    nc = bass.Bass()
    wd_d = nc.declare_dram_parameter("wd", [nlg, KC, nig * PI], BF16, False)
    zs_d = nc.declare_dram_parameter("zs", [nkc, KC, BC], BF16, False)
    coefs_d = nc.declare_dram_parameter("coefs", [nkc, KC, B], BF16, False)
    out_d = nc.declare_dram_parameter("out", [PI, 1 + nit], F32, True)

    with tile.TileContext(nc) as tc, ExitStack() as ctx:
        const_pool = ctx.enter_context(tc.tile_pool(name="const", bufs=1))
        es_pool = ctx.enter_context(tc.tile_pool(name="es", bufs=2))
        i16_pool = ctx.enter_context(tc.tile_pool(name="i16", bufs=2))
        h_pool = ctx.enter_context(tc.tile_pool(name="h", bufs=2))
        small = ctx.enter_context(tc.tile_pool(name="small", bufs=1))
        psum = ctx.enter_context(tc.tile_pool(name="psum", bufs=2, space="PSUM"))

        # --- persistent loads ---
        zs_t, coefs_t, wd_t = [], [], []
        for k in range(nkc):
            t2 = const_pool.tile([KC, B], BF16, tag=f"cs{k}", name=f"cs{k}")
            nc.sync.dma_start(out=t2[:], in_=coefs_d[k])
            coefs_t.append(t2)
            t = const_pool.tile([KC, BC], BF16, tag=f"zs{k}", name=f"zs{k}")
            nc.sync.dma_start(out=t[:], in_=zs_d[k])
            zs_t.append(t)
        for lg in range(nlg):
            t = const_pool.tile([KC, nig * PI], BF16, tag=f"wd{lg}",
                                name=f"wd{lg}")
            nc.sync.dma_start(out=t[:], in_=wd_d[lg])
            wd_t.append(t)

        g_all = small.tile([PI, ntiles], F32, tag="gall", name="gall")
        lq_t = {}
        biasb = small.tile([PI, 1], F32, tag="biasb")
        nc.gpsimd.memset(biasb[:], -ENC_B * ENC_C)

        def phase_a(it):
            # log_qz: S = sum_l y_l = ENC_A * (sum_l arg_l) + L*ENC_B
            sp = psum.tile([PI, B], F32, tag="ring", name=f"sp{it}")
            for k in range(nkc):
                lhsT = zs_t[k][:, it * PI:(it + 1) * PI]
                for jc in range(njc):
                    nc.tensor.matmul(
                        sp[:, jc * JT:(jc + 1) * JT],
                        lhsT,
                        coefs_t[k][:, jc * JT:(jc + 1) * JT],
                        start=(k == 0),
                        stop=(k == nkc - 1),
                    )
            mx = small.tile([PI, 1], F32, tag=f"mx{it}", name=f"mx{it}")
            nc.vector.tensor_reduce(mx[:], sp[:], axis=mybir.AxisListType.X,
                                    op=ALU.max)
            negmxc = small.tile([PI, 1], F32, tag=f"negmxc{it}",
                                name=f"negmxc{it}")
            nc.scalar.mul(negmxc[:], mx[:], -ENC_C)
            es = es_pool.tile([PI, B], F32, tag="es", name=f"esA{it}")
            sume = small.tile([PI, 1], F32, tag=f"sume{it}", name=f"sume{it}")
            nc.scalar.activation(es[:], sp[:], AF.Exp, bias=negmxc[:],
                                 scale=ENC_C, accum_out=sume[:])
            lq = small.tile([PI, 1], F32, tag=f"lq{it}", name=f"lq{it}")
            nc.scalar.activation(lq[:], sume[:], AF.Ln)
            mxc = small.tile([PI, 1], F32, tag=f"mxc{it}", name=f"mxc{it}")
            nc.scalar.mul(mxc[:], mx[:], ENC_C)
            nc.vector.tensor_add(lq[:], lq[:], mxc[:])
            lq_t[it] = lq
            # lq is short by L*ENC_B*ENC_C vs ln(sum_j exp(S)); host corrects.

        # --- phase B: G[(ls,is), tile] = sum_j exp(arg) ---
        k_flat = 0
        for lg in range(nlg):
            for ig in range(nig):
                if k_flat == PHASEA_AT:
                    for it2 in range(nit):
                        phase_a(it2)
                ap = psum.tile([PI, B], F32, tag="ring")
                lhsT = wd_t[lg][:, ig * PI:(ig + 1) * PI]
                for jc in range(njc):
                    nc.tensor.matmul(
                        ap[:, jc * JT:(jc + 1) * JT],
                        lhsT,
                        coefs_t[lg][:, jc * JT:(jc + 1) * JT],
                        start=True,
                        stop=True,
                    )
                gcol = g_all[:, k_flat:k_flat + 1]
                if is_dve(k_flat):
                    e16 = i16_pool.tile([PI, B], I16, tag="e16")
                    nc.vector.tensor_scalar(e16[:], ap[:], SIG, 0.0,
                                            ALU.add, ALU.max)
                    ef = e16[:].bitcast(F16)
                    if GPSIMD_HALVE:
                        h = h_pool.tile([PI, B // 2], F16, tag="h")
                        nc.gpsimd.tensor_tensor(
                            h[:], e16[:, :B // 2].bitcast(F16),
                            e16[:, B // 2:].bitcast(F16), ALU.add)
                        ef = h[:]
                    nc.vector.tensor_reduce(gcol, ef,
                                            axis=mybir.AxisListType.X,
                                            op=ALU.add)
                else:
                    ed = es_pool.tile([PI, B], BF16, tag="ed")
                    nc.scalar.activation(ed[:], ap[:], AF.Exp,
                                         bias=biasb[:], scale=ENC_C,
                                         accum_out=gcol)
                k_flat += 1

        # --- combine: ln(G), free-reduce; DMA per-partition partials ---
        logg = small.tile([PI, ntiles], F32, tag="logg")
        nc.scalar.activation(logg[:], g_all[:], AF.Ln)
        res = small.tile([PI, 1 + nit], F32, tag="res")
        nc.vector.tensor_reduce(res[:, 0:1], logg[:],
                                axis=mybir.AxisListType.X, op=ALU.add)
        for it in range(nit):
            nc.vector.tensor_copy(res[:, 1 + it:2 + it], lq_t[it][:])
        nc.sync.dma_start(out=out_d[:], in_=res[:])

    return _split_multi_waits(nc) if split_waits else nc


def _split_multi_waits(nc):
    """Walrus (gen3 codegen) accepts at most ONE sync-wait per instruction.
    Tile's wait assignment can attach several. Split the extras onto NoOp
    instructions on the same engine immediately before the instruction —
    same-engine streams execute in order, so semantics are preserved."""
    wid = [0]

    def fix_block(b):
        new = []
        for inst in b.instructions:
            si = inst.sync_info
            if si is not None and si.on_wait and len(si.on_wait) > 1:
                for w in si.on_wait[:-1]:
                    wid[0] += 1
                    nop = mybir.InstNoOp(
                        name=f"WSPLIT-{wid[0]}",
                        engine=inst.engine,
                        sync_info=mybir.SyncInfo(on_wait=[w], on_update=[]),
                    )
                    nop.bass_nofuse = True
                    new.append(nop)
                si.on_wait = [si.on_wait[-1]]
            new.append(inst)
        b.instructions[:] = new

    for fn in nc.m.functions:
        for b in fn.blocks:
            fix_block(b)
    return nc


def make_inputs(kl, z_mean, z_logvar, z_sampled, n_cores):
    """Host-side O(B*L) prep: y-encoded coefficients + block-diag z."""
    B, L = kl.shape
    BC = B // n_cores
    PI = 128
    KC = 3 * LG
    nkc = (3 * L) // KC
    nlg = L // LG
    nig = BC // IG

    m = np.asarray(z_mean, dtype=np.float32)
    v = np.asarray(z_logvar, dtype=np.float32)
    z = np.asarray(z_sampled, dtype=np.float32)

    w = np.exp(-v)
    a = ENC_A * (-0.5 * w)
    b = ENC_A * (w * m)
    g = ENC_A * (-0.5 * (w * m * m + v + LOG_2PI)) + ENC_B
    import ml_dtypes
    bf = ml_dtypes.bfloat16
    coefs = np.ascontiguousarray(
        np.stack([a, b, g], 0).transpose(2, 0, 1).reshape(3 * L, B)
        .reshape(nkc, KC, B)).astype(bf)  # [nkc, KC, B], row = l*3+k

    rng = np.random.default_rng(12345)

    in_maps = []
    for c in range(n_cores):
        zc = z[c * BC:(c + 1) * BC]                      # [BC, L]
        arr = np.stack([zc * zc, zc, np.ones_like(zc)], 0)  # [3, BC, L]
        zs = np.ascontiguousarray(
            arr.transpose(2, 0, 1).reshape(3 * L, BC)
            .reshape(nkc, KC, BC)).astype(bf)
        # block-diagonal stationaries: wd[lg, ls*3+k, ig*PI + ls*IG+is]
        # = arr[k, ig*IG+is, lg*LG+ls]; off-block tiny noise keeps the
        # PE power/activity governor at the high clock p-state.
        wd = (rng.integers(0, 2, size=(nlg, KC, nig * PI)) * 2e-30 - 1e-30
              ).astype(np.float32)
        ls_arr = np.arange(LG)
        for lg in range(nlg):
            blk = arr[:, :, lg * LG:(lg + 1) * LG]       # [3, BC, LG]
            # rows ls*3+k ; cols ig*PI + ls*IG + is
            for k in range(3):
                rows = ls_arr * 3 + k                     # [LG]
                colbase = np.arange(nig)[:, None] * PI + ls_arr[None, :] * IG
                for is_ in range(IG):
                    cols = colbase + is_                  # [nig, LG]
                    ivals = blk[k, np.arange(nig)[:, None] * IG + is_, ls_arr[None, :]]
                    wd[lg, rows[None, :].repeat(nig, 0), cols] = ivals
        in_maps.append({
            "wd": np.ascontiguousarray(wd).astype(bf),
            "zs": zs,
            "coefs": coefs,
        })
    return in_maps


_NC_CACHE = {}


def _get_nc(B, L, BC):
    key = (B, L, BC)
    if key not in _NC_CACHE:
        _NC_CACHE[key] = build_nc(B, L, BC)
    return _NC_CACHE[key]


def _enable_jax_cache():
    try:
        import jax
        jax.config.update("jax_compilation_cache_dir", "/tmp/jaxcache")
        jax.config.update("jax_persistent_cache_min_entry_size_bytes", 0)
        jax.config.update("jax_persistent_cache_min_compile_time_secs", 0)
    except Exception:
        pass


def host_total(results, kl, B, L):
    """Combine per-core per-partition partials on host."""
    scale_r = (BETA - 1.0) / float(B)
    tot = 0.0
    for r in results:
        o = np.asarray(r["out"], dtype=np.float64)
        sum_lng = o[:, 0].sum()          # sum_{i,l in core} ln G
        sum_lq = o[:, 1:].sum()          # sum_i lq (encoded)
        tot += scale_r * (sum_lq - sum_lng)
    tot -= (BETA - 1.0) * (L * ENC_B * ENC_C)   # lq encoding offset
    tot += float(np.asarray(kl, dtype=np.float64).sum())
    return np.float32(tot)


def kernel(kl, z_mean, z_logvar, z_sampled):
    from concourse.bass_utils import run_bass_kernel_spmd

    _enable_jax_cache()

    B, L = kl.shape
    n_cores = 8
    BC = B // n_cores
    nc = _get_nc(B, L, BC)
    in_maps = make_inputs(kl, z_mean, z_logvar, z_sampled, n_cores)
    res = run_bass_kernel_spmd(nc, in_maps, list(range(n_cores)))
    return host_total(res.results, kl, B, L)
